# revision 26
# baseline (speedup 1.0000x reference)
"""Megatron-style MoE layer (precomputed routing) on 8 Trainium2 NeuronCores.

Strategy: expert x FFN-slice parallelism ("slot" sharding). Each expert's
FFN hidden dim F=4096 is split into 8 slices of 512; core i owns slice i of
EVERY expert. The kernel runs 8 sequential "slots" (slot j = expert j); in
slot j every core processes the SAME token set (expert j's tokens, exact
count, no padding) against its own F-slice of w1/w2:

    y_partial = gelu_tanh(x_e @ w1[e][:, sl]) @ w2[e][sl, :]

All cores therefore execute an identical instruction schedule on identical
column counts -> perfectly balanced SPMD with zero padding waste (the old
per-expert-per-core layout padded every core to the max expert count).
The host sums the 8 partial outputs, applies the gate coefficient, and
scatters pairs back (all f32).

Device layouts (per core), P = 128 partitions:
  xT   [P, 8, CT]   bf16  x^T of all experts' tokens, h = ko*128 + p
  w1   [P, 8, 4096] bf16  slot j at cols j*512:(j+1)*512; lhsT for fc1
  w2   [P, 32, 1024]bf16  slot j at rows j*4:(j+1)*4; f = kf*128 + p (lhsT fc2)
  y    [P, 8, CT]   bf16  y^T partial, hh = mh*128 + p

Extras: a short stream of dummy matmuls on zeroed scratch warms the PE HAM
clock gate (PE defaults to 1.2 GHz; sustained activity unlocks 2.4 GHz)
during the unavoidable DMA preamble, and the first slot's weights/x are
DMA'd in fine-grained ko-interleaved chunks so the first real matmul can
issue as early as possible.

DMA queue discipline (hard-won): a dma_start instruction occupies its
issuing engine for 0.6-8us (descriptor setup + any inline semaphore wait),
so (a) LOADS (never wait) go on the scalar HWDGE queue, where they can
never block behind a store's wait and cost the scalar engine ~1us/tile;
(b) STORES (which wait on the psum-drain casts) go alone on the sync
queue, batched ONE per tile ([P, 8, N], casts write slices) so the sync
engine issues ~3 instead of ~10 blocking instructions per tile. Putting
stores on the scalar queue instead starves gelu (strict FIFO -> 184us of
PE gaps); leaving everything on sync starves x loads behind store waits
(~100us of PE gaps via psum-recycle back-pressure).
"""

import sys
import numpy as np
import ml_dtypes


def _ensure_axon_hooks():
    """bass_utils imports antenv.axon_hooks when BASS_TRACE is set; this
    image ships an antenv stub without it. Provide a working (or None)
    hook so tracing requests degrade gracefully instead of crashing."""
    try:
        import antenv.axon_hooks  # noqa: F401
        return
    except ImportError:
        pass
    import os
    import types

    mod = types.ModuleType("antenv.axon_hooks")
    state = [None]

    def set_axon_ntff_profile_hook(h):
        state[0] = h

    def get_axon_ntff_profile_hook():
        if state[0] is None:
            try:
                from trn_agent_boot.trn_boot import _ntff_profile_via_ctypes
                so = os.environ.get("PJRT_LIBRARY_PATH",
                                    "/opt/axon/libaxon_pjrt.so")
                if os.path.exists(so):
                    state[0] = _ntff_profile_via_ctypes(so)
            except Exception:
                pass
        return state[0]

    mod.set_axon_ntff_profile_hook = set_axon_ntff_profile_hook
    mod.get_axon_ntff_profile_hook = get_axon_ntff_profile_hook
    sys.modules["antenv.axon_hooks"] = mod
    try:
        import antenv
        antenv.axon_hooks = mod
    except ImportError:
        pass
    try:
        from concourse import bass_utils as _bu
        _orig = _bu.upload_artifacts

        def _safe_upload(tmpdir):
            try:
                return _orig(tmpdir)
            except Exception:
                return "local://" + tmpdir

        _bu.upload_artifacts = _safe_upload
    except Exception:
        pass


S, B, H = 1024, 8, 1024
T = S * B
E, K, F = 8, 2, 4096
P = 128
NCORES = 8
FS = F // NCORES          # F-slice per core (512)
MF = FS // P              # fc1 output 128-groups per slot (4)
NDUMMY = 6               # warm-up matmuls on scratch during DMA preamble

_CACHE: dict[tuple, object] = {}

TRACE = False
LAST_RESULTS = None


def _tile_sizes(c: int, small_last: bool = False,
                small_first: bool = False) -> list[int]:
    """Split c columns into ceil(c/512) near-equal tiles (exact cover).
    small_last carves a ~256-col final tile (short end-of-kernel drain);
    small_first carves a 128-col lead tile (first matmul group only needs
    ~512KB of DMA, beating the slow early DMA ramp)."""
    if c == 0:
        return []
    if small_first and c > 512:
        return [128] + _tile_sizes(c - 128, small_last=small_last)
    if small_last and c > 512:
        last = 256
        head = _tile_sizes(c - last)
        if max(head) <= 512:
            return head + [last]
    nt = -(-c // 512)
    base = c // nt
    rem = c - base * nt
    return [base + (1 if i < rem else 0) for i in range(nt)]


def _build(counts: tuple):
    import concourse.bacc as bacc
    import concourse.mybir as mybir
    import concourse.tile as tile

    dt = mybir.dt
    AF = mybir.ActivationFunctionType

    CT = int(sum(counts))
    offs = np.concatenate([[0], np.cumsum(counts)]).astype(int)

    nc = bacc.Bacc("TRN2", target_bir_lowering=False, debug=False,
                   num_devices=NCORES)

    xT_d = nc.dram_tensor("xT", [P, 8, CT], dt.bfloat16, kind="ExternalInput").ap()
    w1_d = nc.dram_tensor("w1", [P, 8, F], dt.bfloat16, kind="ExternalInput").ap()
    w2_d = nc.dram_tensor("w2", [P, 32, H], dt.bfloat16, kind="ExternalInput").ap()
    y_d = nc.dram_tensor("y", [P, 8, CT], dt.bfloat16, kind="ExternalOutput").ap()

    with tile.TileContext(nc) as tc:
        with (
            tc.tile_pool(name="wpool", bufs=1) as wpool,
            tc.tile_pool(name="spool", bufs=1) as spool,
            tc.tile_pool(name="xpool", bufs=4) as xpool,
            tc.tile_pool(name="hpool", bufs=2) as hpool,
            tc.tile_pool(name="opool", bufs=3) as opool,
            tc.tile_pool(name="ps1", bufs=3, space="PSUM") as ps1,
            tc.tile_pool(name="ps2", bufs=4, space="PSUM") as ps2,
        ):
            w1_sb = wpool.tile([P, 8, F], dt.bfloat16, tag="w1")
            w2_sb = wpool.tile([P, 32, H], dt.bfloat16, tag="w2")

            # --- PE warm-up: dummy matmuls on zeroed scratch keep the PE
            # active through the DMA preamble (and, interspersed, through
            # the slow early-DMA trickle) so the HAM clock gate reaches and
            # holds 8/8 (2.4 GHz). Results are never read.
            scratch = spool.tile([P, 640], dt.bfloat16, tag="scratch")
            nc.vector.memset(scratch[:, :], 0.0)

            def dummy_mms(n):
                for _ in range(n):
                    pd = ps1.tile([P, 512], dt.float32, tag="p1")
                    nc.tensor.matmul(pd[:, :], scratch[:, 0:128],
                                     scratch[:, 128:640], start=True,
                                     stop=True)

            dummy_mms(NDUMMY)

            # global tile list: (slot j, col offset within the full xT, N)
            tiles = []
            for j in range(NCORES):
                t0 = 0
                for N in _tile_sizes(counts[j], small_last=(j == NCORES - 1),
                                     small_first=(j == 0)):
                    tiles.append((j, int(offs[j]) + t0, N))
                    t0 += N
            ntiles = len(tiles)

            def load_x(g, queue=None):
                # alternate queues by tile parity: halves the per-queue DMA
                # ring pressure (a load issued into a full ring blocks its
                # engine for multi-us; on scalar that starves gelu)
                jg, c0, Ng = tiles[g]
                q = queue or (nc.sync if g % 2 == 0 else nc.scalar)
                xt = xpool.tile([P, 8, 512], dt.bfloat16, tag="x")
                q.dma_start(xt[:, :, :Ng], xT_d[:, :, c0:c0 + Ng])
                return xt

            # startup: the early DMA system ramps slowly (~170GB/s for the
            # first ~5us), so sequence the two queues to give the lead
            # 128-col tile's first matmul group its ~512KB ASAP, then feed
            # strictly in consumption order
            xt0 = xpool.tile([P, 8, 512], dt.bfloat16, tag="x")
            _, c00, N00 = tiles[0]
            nc.sync.dma_start(w1_sb[:, :, 0:128], w1_d[:, :, 0:128])
            nc.scalar.dma_start(xt0[:, :, :N00], xT_d[:, :, c00:c00 + N00])
            nc.sync.dma_start(w1_sb[:, :, 128:FS], w1_d[:, :, 128:FS])
            x_tiles = {0: xt0}
            if ntiles > 1:
                x_tiles[1] = load_x(1, queue=nc.scalar)
            nc.sync.dma_start(w2_sb[:, 0:4, 0:512], w2_d[:, 0:4, 0:512])
            nc.sync.dma_start(w2_sb[:, 0:4, 512:H], w2_d[:, 0:4, 512:H])
            if ntiles > 2:
                x_tiles[2] = load_x(2, queue=nc.scalar)

            def fc2_group(prev, mh, ot, drain_engine):
                """One fc2 output group of the previous tile: 4-deep psum
                accumulation, drain psum -> slice of the tile's out buffer."""
                jp, cp, Np, hp = prev
                p2 = ps2.tile([P, 512], dt.float32, tag="p2")
                for kf in range(MF):
                    nc.tensor.matmul(
                        p2[:, :Np],
                        w2_sb[:, jp * 4 + kf, mh * 128:(mh + 1) * 128],
                        hp[:, kf, :Np],
                        start=(kf == 0), stop=(kf == MF - 1),
                    )
                if drain_engine == "scalar":
                    nc.scalar.activation(ot[:, mh, :Np], p2[:, :Np], AF.Copy)
                else:
                    nc.vector.tensor_copy(ot[:, mh, :Np], p2[:, :Np])

            # fc2 of tile g-1 is software-pipelined into the fc1 phase of
            # tile g: its 8 psum drains spread across the whole tile span
            # instead of bunching in the fc2 phase (the 4-deep fc2 groups
            # complete every ~850ns -- faster than one engine can drain).
            prev = None
            ot_prev = None
            for g, (j, c0, N) in enumerate(tiles):
                xt = x_tiles.pop(g)

                h = hpool.tile([P, MF, 512], dt.bfloat16, tag="h")
                for mf in range(MF):
                    if prev is not None:
                        fc2_group(prev, 2 * mf, ot_prev, "vector")
                        fc2_group(prev, 2 * mf + 1, ot_prev, "vector")
                        if mf == MF - 1:
                            # single batched store for the whole prev tile
                            jp, cp, Np, _ = prev
                            nc.sync.dma_start(y_d[:, :, cp:cp + Np],
                                              ot_prev[:, :, :Np])
                    p1 = ps1.tile([P, 512], dt.float32, tag="p1")
                    f0 = j * FS + mf * 128
                    for ko in range(8):
                        nc.tensor.matmul(
                            p1[:, :N],
                            w1_sb[:, ko, f0:f0 + 128],
                            xt[:, ko, :N],
                            start=(ko == 0), stop=(ko == 7),
                        )
                    nc.scalar.activation(h[:, mf, :N], p1[:, :N],
                                         AF.Gelu_apprx_tanh)
                    if g == 0:
                        # bridge the early-DMA trickle between data-gated
                        # groups of the lead tile (idle >3.4us re-throttles
                        # the PE clock to 1.2GHz)
                        dummy_mms(2)

                # DMA issues at the BOTTOM of the body: a load issued into
                # a full ring blocks the issuing engine for multi-us; down
                # here nothing latency-critical sits behind it
                if g + 2 < ntiles:
                    x_tiles[g + 2] = load_x(g + 2)
                if (g == 0 or j != tiles[g - 1][0]) and j + 1 < NCORES \
                        and counts[j + 1]:
                    # prefetch next slot's weights during this slot
                    jn = j + 1
                    nc.scalar.dma_start(w1_sb[:, :, jn * FS:(jn + 1) * FS],
                                        w1_d[:, :, jn * FS:(jn + 1) * FS])
                    nc.scalar.dma_start(w2_sb[:, jn * 4:(jn + 1) * 4, :],
                                        w2_d[:, jn * 4:(jn + 1) * 4, :])
                prev = (j, c0, N, h)
                ot_prev = opool.tile([P, 8, 512], dt.bfloat16, tag="o")

            # final tile's fc2 has no successor to hide in: alternate the
            # drains across scalar+vector; store in two batched halves
            # (per-group stores cost ~600ns of serialized sync-engine issue
            # time EACH after the last matmul)
            jp, cp, Np, _ = prev
            for mh in range(8):
                fc2_group(prev, mh, ot_prev, "scalar" if mh % 2 else "vector")
                if mh == 3:
                    nc.sync.dma_start(y_d[:, 0:4, cp:cp + Np],
                                      ot_prev[:, 0:4, :Np])
            nc.sync.dma_start(y_d[:, 4:8, cp:cp + Np], ot_prev[:, 4:8, :Np])

    nc.compile()
    return nc


def kernel(hidden_states, gate_weight, choosed_experts, w1, w2):
    global LAST_RESULTS
    _ensure_axon_hooks()
    from concourse import bass_utils

    x = np.asarray(hidden_states, dtype=np.float32).reshape(T, H)
    gw = np.asarray(gate_weight, dtype=np.float32)
    ce = np.asarray(choosed_experts).astype(np.int64)
    w1 = np.asarray(w1, dtype=np.float32)
    w2 = np.asarray(w2, dtype=np.float32)

    # routing: stable sort of (token, k) pairs by expert
    flat = ce.reshape(-1)
    order = np.argsort(flat, kind="stable")
    counts = np.bincount(flat, minlength=E).astype(np.int64)
    starts = np.zeros(E + 1, dtype=np.int64)
    starts[1:] = np.cumsum(counts)
    CT = int(counts.sum())

    key = tuple(int(c) for c in counts)
    nc = _CACHE.get(key)
    if nc is None:
        nc = _build(key)
        _CACHE[key] = nc

    bf16 = ml_dtypes.bfloat16

    # xT for ALL pairs in expert order: [H, CT] -> [P, 8, CT]; identical on
    # every core (each core consumes a different F-slice of the weights).
    t_idx_all = order // K
    k_idx_all = order % K
    xT = np.ascontiguousarray(
        x[t_idx_all].T.astype(bf16).reshape(8, P, CT).transpose(1, 0, 2))

    in_maps = []
    for i in range(NCORES):
        sl = slice(i * FS, (i + 1) * FS)
        # w1 slice: [H, FS] per expert -> [P, 8ko, FS] blocks side by side
        w1_i = np.empty((P, 8, F), dtype=bf16)
        w2_i = np.empty((P, 32, H), dtype=bf16)
        for e in range(E):
            w1_i[:, :, e * FS:(e + 1) * FS] = (
                w1[e][:, sl].astype(bf16).reshape(8, P, FS).transpose(1, 0, 2))
            w2_i[:, e * 4:(e + 1) * 4, :] = (
                w2[e][sl, :].astype(bf16).reshape(4, P, H).transpose(1, 0, 2))
        in_maps.append({"xT": xT, "w1": np.ascontiguousarray(w1_i),
                        "w2": np.ascontiguousarray(w2_i)})

    res = bass_utils.run_bass_kernel_spmd(nc, in_maps, list(range(NCORES)),
                                          trace=TRACE)
    LAST_RESULTS = res

    # combine: sum the 8 F-slice partials, apply gate coef, scatter back
    yT = np.zeros((H, CT), dtype=np.float32)
    for i in range(NCORES):
        yi = np.asarray(res.results[i]["y"], dtype=np.float32)  # [P, 8, CT]
        yT += yi.transpose(1, 0, 2).reshape(H, CT)
    coef = gw[t_idx_all, k_idx_all].astype(np.float32)
    all_pairs = yT.T * coef[:, None]  # [CT, H] in expert order
    out_pairs = np.empty((T * K, H), dtype=np.float32)
    out_pairs[order] = all_pairs
    return out_pairs.reshape(T, K, H).sum(axis=1)


# revision 27
# speedup vs baseline: 1.0787x; 1.0787x over previous
"""Megatron-style MoE layer (precomputed routing) on 8 Trainium2 NeuronCores.

Strategy: expert x FFN-slice parallelism ("slot" sharding). Each expert's
FFN hidden dim F=4096 is split into 8 slices of 512; core i owns slice i of
EVERY expert. The kernel runs 8 sequential "slots" (slot j = expert j); in
slot j every core processes the SAME token set (expert j's tokens, exact
count, no padding) against its own F-slice of w1/w2:

    y_partial = gelu_tanh(x_e @ w1[e][:, sl]) @ w2[e][sl, :]

All cores therefore execute an identical instruction schedule on identical
column counts -> perfectly balanced SPMD with zero padding waste (the old
per-expert-per-core layout padded every core to the max expert count).
The host sums the 8 partial outputs, applies the gate coefficient, and
scatters pairs back (all f32).

Device layouts (per core), P = 128 partitions:
  xT   [P, 8, CT]   bf16  x^T of all experts' tokens, h = ko*128 + p
  w1   [P, 8, 4096] bf16  slot j at cols j*512:(j+1)*512; lhsT for fc1
  w2   [P, 32, 1024]bf16  slot j at rows j*4:(j+1)*4; f = kf*128 + p (lhsT fc2)
  y    [P, 8, CT]   bf16  y^T partial, hh = mh*128 + p

Extras: a short stream of dummy matmuls on zeroed scratch warms the PE HAM
clock gate (PE defaults to 1.2 GHz; sustained activity unlocks 2.4 GHz)
during the unavoidable DMA preamble, and the first slot's weights/x are
DMA'd in fine-grained ko-interleaved chunks so the first real matmul can
issue as early as possible.

DMA queue discipline (hard-won): a dma_start instruction occupies its
issuing engine for 0.6-8us (descriptor setup + any inline semaphore wait),
so (a) LOADS (never wait) go on the scalar HWDGE queue, where they can
never block behind a store's wait and cost the scalar engine ~1us/tile;
(b) STORES (which wait on the psum-drain casts) go alone on the sync
queue, batched ONE per tile ([P, 8, N], casts write slices) so the sync
engine issues ~3 instead of ~10 blocking instructions per tile. Putting
stores on the scalar queue instead starves gelu (strict FIFO -> 184us of
PE gaps); leaving everything on sync starves x loads behind store waits
(~100us of PE gaps via psum-recycle back-pressure).
"""

import sys
import numpy as np
import ml_dtypes


def _ensure_axon_hooks():
    """bass_utils imports antenv.axon_hooks when BASS_TRACE is set; this
    image ships an antenv stub without it. Provide a working (or None)
    hook so tracing requests degrade gracefully instead of crashing."""
    try:
        import antenv.axon_hooks  # noqa: F401
        return
    except ImportError:
        pass
    import os
    import types

    mod = types.ModuleType("antenv.axon_hooks")
    state = [None]

    def set_axon_ntff_profile_hook(h):
        state[0] = h

    def get_axon_ntff_profile_hook():
        if state[0] is None:
            try:
                from trn_agent_boot.trn_boot import _ntff_profile_via_ctypes
                so = os.environ.get("PJRT_LIBRARY_PATH",
                                    "/opt/axon/libaxon_pjrt.so")
                if os.path.exists(so):
                    state[0] = _ntff_profile_via_ctypes(so)
            except Exception:
                pass
        return state[0]

    mod.set_axon_ntff_profile_hook = set_axon_ntff_profile_hook
    mod.get_axon_ntff_profile_hook = get_axon_ntff_profile_hook
    sys.modules["antenv.axon_hooks"] = mod
    try:
        import antenv
        antenv.axon_hooks = mod
    except ImportError:
        pass
    try:
        from concourse import bass_utils as _bu
        _orig = _bu.upload_artifacts

        def _safe_upload(tmpdir):
            try:
                return _orig(tmpdir)
            except Exception:
                return "local://" + tmpdir

        _bu.upload_artifacts = _safe_upload
    except Exception:
        pass


S, B, H = 1024, 8, 1024
T = S * B
E, K, F = 8, 2, 4096
P = 128
NCORES = 8
FS = F // NCORES          # F-slice per core (512)
MF = FS // P              # fc1 output 128-groups per slot (4)
NDUMMY = 6               # warm-up matmuls on scratch during DMA preamble

_CACHE: dict[tuple, object] = {}

TRACE = False
LAST_RESULTS = None


def _tile_sizes(c: int, small_last: bool = False,
                small_first: bool = False) -> list[int]:
    """Split c columns into ceil(c/512) near-equal tiles (exact cover).
    small_last carves a ~256-col final tile (short end-of-kernel drain);
    small_first carves a 128-col lead tile (first matmul group only needs
    ~512KB of DMA, beating the slow early DMA ramp)."""
    if c == 0:
        return []
    if small_first and c > 512:
        return [128] + _tile_sizes(c - 128, small_last=small_last)
    if small_last and c > 512:
        last = 256
        head = _tile_sizes(c - last)
        if max(head) <= 512:
            return head + [last]
    nt = -(-c // 512)
    base = c // nt
    rem = c - base * nt
    return [base + (1 if i < rem else 0) for i in range(nt)]


def _build(counts: tuple):
    import concourse.bacc as bacc
    import concourse.mybir as mybir
    import concourse.tile as tile

    dt = mybir.dt
    AF = mybir.ActivationFunctionType

    CT = int(sum(counts))
    offs = np.concatenate([[0], np.cumsum(counts)]).astype(int)

    nc = bacc.Bacc("TRN2", target_bir_lowering=False, debug=False,
                   num_devices=NCORES)

    xT_d = nc.dram_tensor("xT", [P, 8, CT], dt.bfloat16, kind="ExternalInput").ap()
    w1_d = nc.dram_tensor("w1", [P, 8, F], dt.bfloat16, kind="ExternalInput").ap()
    w2_d = nc.dram_tensor("w2", [P, 32, H], dt.bfloat16, kind="ExternalInput").ap()
    y_d = nc.dram_tensor("y", [P, 8, CT], dt.bfloat16, kind="ExternalOutput").ap()

    with tile.TileContext(nc) as tc:
        with (
            tc.tile_pool(name="wpool", bufs=1) as wpool,
            tc.tile_pool(name="spool", bufs=1) as spool,
            tc.tile_pool(name="xpool", bufs=4) as xpool,
            tc.tile_pool(name="hpool", bufs=2) as hpool,
            tc.tile_pool(name="opool", bufs=3) as opool,
            tc.tile_pool(name="ps1", bufs=3, space="PSUM") as ps1,
            tc.tile_pool(name="ps2", bufs=4, space="PSUM") as ps2,
        ):
            w1_sb = wpool.tile([P, 8, F], dt.bfloat16, tag="w1")
            w2_sb = wpool.tile([P, 32, H], dt.bfloat16, tag="w2")

            # --- PE warm-up: dummy matmuls on zeroed scratch keep the PE
            # active through the DMA preamble (and, interspersed, through
            # the slow early-DMA trickle) so the HAM clock gate reaches and
            # holds 8/8 (2.4 GHz). Results are never read.
            scratch = spool.tile([P, 640], dt.bfloat16, tag="scratch")
            nc.vector.memset(scratch[:, :], 0.0)

            def dummy_mms(n):
                for _ in range(n):
                    pd = ps1.tile([P, 512], dt.float32, tag="p1")
                    nc.tensor.matmul(pd[:, :], scratch[:, 0:128],
                                     scratch[:, 128:640], start=True,
                                     stop=True)

            dummy_mms(NDUMMY)

            # global tile list: (slot j, col offset within the full xT, N)
            tiles = []
            for j in range(NCORES):
                t0 = 0
                for N in _tile_sizes(counts[j], small_last=(j == NCORES - 1),
                                     small_first=(j == 0)):
                    tiles.append((j, int(offs[j]) + t0, N))
                    t0 += N
            ntiles = len(tiles)

            def load_x(g, queue=None):
                # alternate queues by tile parity: halves the per-queue DMA
                # ring pressure (a load issued into a full ring blocks its
                # engine for multi-us; on scalar that starves gelu)
                jg, c0, Ng = tiles[g]
                q = queue or (nc.sync if g % 2 == 0 else nc.scalar)
                xt = xpool.tile([P, 8, 512], dt.bfloat16, tag="x")
                q.dma_start(xt[:, :, :Ng], xT_d[:, :, c0:c0 + Ng])
                return xt

            # startup: the early DMA system ramps slowly (~170GB/s for the
            # first ~5us), so sequence the two queues to give the lead
            # 128-col tile's first matmul group its ~512KB ASAP, then feed
            # strictly in consumption order
            xt0 = xpool.tile([P, 8, 512], dt.bfloat16, tag="x")
            _, c00, N00 = tiles[0]
            nc.sync.dma_start(w1_sb[:, :, 0:128], w1_d[:, :, 0:128])
            nc.scalar.dma_start(xt0[:, :, :N00], xT_d[:, :, c00:c00 + N00])
            nc.sync.dma_start(w1_sb[:, :, 128:FS], w1_d[:, :, 128:FS])
            x_tiles = {0: xt0}
            if ntiles > 1:
                x_tiles[1] = load_x(1, queue=nc.scalar)
            nc.sync.dma_start(w2_sb[:, 0:4, 0:512], w2_d[:, 0:4, 0:512])
            nc.sync.dma_start(w2_sb[:, 0:4, 512:H], w2_d[:, 0:4, 512:H])
            if ntiles > 2:
                x_tiles[2] = load_x(2, queue=nc.scalar)

            def fc2_group(prev, mh, ot, drain_engine):
                """One fc2 output group of the previous tile: 4-deep psum
                accumulation, drain psum -> slice of the tile's out buffer."""
                jp, cp, Np, hp = prev
                p2 = ps2.tile([P, 512], dt.float32, tag="p2")
                for kf in range(MF):
                    nc.tensor.matmul(
                        p2[:, :Np],
                        w2_sb[:, jp * 4 + kf, mh * 128:(mh + 1) * 128],
                        hp[:, kf, :Np],
                        start=(kf == 0), stop=(kf == MF - 1),
                    )
                if drain_engine == "scalar":
                    nc.scalar.activation(ot[:, mh, :Np], p2[:, :Np], AF.Copy)
                else:
                    nc.vector.tensor_copy(ot[:, mh, :Np], p2[:, :Np])

            # fc2 of tile g-1 is software-pipelined into the fc1 phase of
            # tile g: its 8 psum drains spread across the whole tile span
            # instead of bunching in the fc2 phase (the 4-deep fc2 groups
            # complete every ~850ns -- faster than one engine can drain).
            prev = None
            ot_prev = None
            for g, (j, c0, N) in enumerate(tiles):
                xt = x_tiles.pop(g)

                h = hpool.tile([P, MF, 512], dt.bfloat16, tag="h")
                for mf in range(MF):
                    if prev is not None:
                        fc2_group(prev, 2 * mf, ot_prev, "vector")
                        fc2_group(prev, 2 * mf + 1, ot_prev, "vector")
                        if mf == MF - 1:
                            # single batched store for the whole prev tile
                            jp, cp, Np, _ = prev
                            nc.sync.dma_start(y_d[:, :, cp:cp + Np],
                                              ot_prev[:, :, :Np])
                    p1 = ps1.tile([P, 512], dt.float32, tag="p1")
                    f0 = j * FS + mf * 128
                    for ko in range(8):
                        nc.tensor.matmul(
                            p1[:, :N],
                            w1_sb[:, ko, f0:f0 + 128],
                            xt[:, ko, :N],
                            start=(ko == 0), stop=(ko == 7),
                        )
                    nc.scalar.activation(h[:, mf, :N], p1[:, :N],
                                         AF.Gelu_apprx_tanh)
                    if g == 0:
                        # bridge the early-DMA trickle between data-gated
                        # groups of the lead tile (idle >3.4us re-throttles
                        # the PE clock to 1.2GHz)
                        dummy_mms(2)

                # DMA issues at the BOTTOM of the body: a load issued into
                # a full ring blocks the issuing engine for multi-us; down
                # here nothing latency-critical sits behind it
                if g + 3 < ntiles and (g + 3) not in x_tiles:
                    x_tiles[g + 3] = load_x(g + 3)
                if (g == 0 or j != tiles[g - 1][0]) and j + 1 < NCORES \
                        and counts[j + 1]:
                    # prefetch next slot's weights during this slot; on the
                    # sync queue (stores-only, light) so the 2MB burst never
                    # crowds x loads out of the scalar queue
                    jn = j + 1
                    nc.sync.dma_start(w1_sb[:, :, jn * FS:(jn + 1) * FS],
                                      w1_d[:, :, jn * FS:(jn + 1) * FS])
                    nc.sync.dma_start(w2_sb[:, jn * 4:(jn + 1) * 4, :],
                                      w2_d[:, jn * 4:(jn + 1) * 4, :])
                prev = (j, c0, N, h)
                ot_prev = opool.tile([P, 8, 512], dt.bfloat16, tag="o")

            # final tile's fc2 has no successor to hide in: alternate the
            # drains across scalar+vector; store in two batched halves
            # (per-group stores cost ~600ns of serialized sync-engine issue
            # time EACH after the last matmul)
            jp, cp, Np, _ = prev
            for mh in range(8):
                fc2_group(prev, mh, ot_prev, "scalar" if mh % 2 else "vector")
                if mh == 3:
                    nc.sync.dma_start(y_d[:, 0:4, cp:cp + Np],
                                      ot_prev[:, 0:4, :Np])
            nc.sync.dma_start(y_d[:, 4:8, cp:cp + Np], ot_prev[:, 4:8, :Np])

    nc.compile()
    return nc


def kernel(hidden_states, gate_weight, choosed_experts, w1, w2):
    global LAST_RESULTS
    _ensure_axon_hooks()
    from concourse import bass_utils

    x = np.asarray(hidden_states, dtype=np.float32).reshape(T, H)
    gw = np.asarray(gate_weight, dtype=np.float32)
    ce = np.asarray(choosed_experts).astype(np.int64)
    w1 = np.asarray(w1, dtype=np.float32)
    w2 = np.asarray(w2, dtype=np.float32)

    # routing: stable sort of (token, k) pairs by expert
    flat = ce.reshape(-1)
    order = np.argsort(flat, kind="stable")
    counts = np.bincount(flat, minlength=E).astype(np.int64)
    starts = np.zeros(E + 1, dtype=np.int64)
    starts[1:] = np.cumsum(counts)
    CT = int(counts.sum())

    key = tuple(int(c) for c in counts)
    nc = _CACHE.get(key)
    if nc is None:
        nc = _build(key)
        _CACHE[key] = nc

    bf16 = ml_dtypes.bfloat16

    # xT for ALL pairs in expert order: [H, CT] -> [P, 8, CT]; identical on
    # every core (each core consumes a different F-slice of the weights).
    t_idx_all = order // K
    k_idx_all = order % K
    xT = np.ascontiguousarray(
        x[t_idx_all].T.astype(bf16).reshape(8, P, CT).transpose(1, 0, 2))

    in_maps = []
    for i in range(NCORES):
        sl = slice(i * FS, (i + 1) * FS)
        # w1 slice: [H, FS] per expert -> [P, 8ko, FS] blocks side by side
        w1_i = np.empty((P, 8, F), dtype=bf16)
        w2_i = np.empty((P, 32, H), dtype=bf16)
        for e in range(E):
            w1_i[:, :, e * FS:(e + 1) * FS] = (
                w1[e][:, sl].astype(bf16).reshape(8, P, FS).transpose(1, 0, 2))
            w2_i[:, e * 4:(e + 1) * 4, :] = (
                w2[e][sl, :].astype(bf16).reshape(4, P, H).transpose(1, 0, 2))
        in_maps.append({"xT": xT, "w1": np.ascontiguousarray(w1_i),
                        "w2": np.ascontiguousarray(w2_i)})

    res = bass_utils.run_bass_kernel_spmd(nc, in_maps, list(range(NCORES)),
                                          trace=TRACE)
    LAST_RESULTS = res

    # combine: sum the 8 F-slice partials, apply gate coef, scatter back
    yT = np.zeros((H, CT), dtype=np.float32)
    for i in range(NCORES):
        yi = np.asarray(res.results[i]["y"], dtype=np.float32)  # [P, 8, CT]
        yT += yi.transpose(1, 0, 2).reshape(H, CT)
    coef = gw[t_idx_all, k_idx_all].astype(np.float32)
    all_pairs = yT.T * coef[:, None]  # [CT, H] in expert order
    out_pairs = np.empty((T * K, H), dtype=np.float32)
    out_pairs[order] = all_pairs
    return out_pairs.reshape(T, K, H).sum(axis=1)
